# revision 1
# baseline (speedup 1.0000x reference)
"""Causal attention block kernel for TRN2, 8 NeuronCores.

Sharding: 8 cores = 4 batches x 2 head-groups (8 heads each).
Each core computes, for its (batch, head-group):
  qkv = x @ w_qkv + b_qkv ; causal softmax attention ; partial out-proj.
Host sums the two head-group partials per batch and adds b_out.

Per-core layout (transposed-score flash attention):
  X^T [d,s] via DMA transpose; Q^T,K^T [64e, 2048s] per head (bf16);
  V natural [s, (h,64e +ones)] (bf16).  Per head, per k-chunk kc:
  S^T[k,q] = K^T.T @ Q^T in PSUM, E = exp(S/8) -> SBUF bf16 (wide ACT ops),
  causal mask on the diagonal 128x128 block, then PV: O'[65,q] += V_aug.T @ E
  accumulated over kc in PSUM (row 64 = softmax denominator via ones column).
  Normalize via reciprocal + selector-matmul partition broadcast, then
  out-proj accumulating over head pairs.
"""

import numpy as np
from contextlib import ExitStack

import concourse.bacc as bacc
import concourse.bass as bass
import concourse.mybir as mybir
import concourse.tile as tile
from concourse import bass_utils

F32 = mybir.dt.float32
F32R = mybir.dt.float32r
BF16 = mybir.dt.bfloat16
AF = mybir.ActivationFunctionType

B, S, D, H, DH = 4, 2048, 1024, 16, 64
HPC = 8            # heads per core
NP = 4             # head pairs per core
NS = S // 128      # 16 s-tiles / k-chunks
NQ = S // 512      # 4 q-blocks
NDC = D // 128     # 8 d-chunks


def _emit(ctx: ExitStack, tc: tile.TileContext, io):
    nc = tc.nc
    x_d, wqk_d, bqk_d, wv_d, bv_d, wo_d, sel_d, tri_d, out_d = io

    const = ctx.enter_context(tc.tile_pool(name="const", bufs=1))

    # ---- resident constants ----
    bqk = const.tile([128, 8], F32, tag="bqk", name="bqk")
    nc.sync.dma_start(bqk[:], bqk_d[:])
    bv = const.tile([128, HPC * DH], F32, tag="bv", name="bv")
    nc.sync.dma_start(bv[:], bv_d[:])
    sel = const.tile([HPC, NP * 128], BF16, tag="sel", name="sel")
    nc.sync.dma_start(sel[:], sel_d[:])
    tri = const.tile([128, 128], BF16, tag="tri", name="tri")
    nc.sync.dma_start(tri[:], tri_d[:])
    wo = [const.tile([128, D], BF16, tag=f"wo{j}", name=f"wo{j}") for j in range(NP)]
    for j in range(NP):
        nc.sync.dma_start(wo[j][:], wo_d[j * 128:(j + 1) * 128, :])

    # X^T [d, s] via 16-bit DMA transpose (x pre-cast to bf16 on host)
    xt = [const.tile([128, S], BF16, tag=f"xt{dc}", name=f"xt{dc}") for dc in range(NDC)]
    for dc in range(NDC):
        nc.sync.dma_start(
            xt[dc][:], x_d[:, dc * 128:(dc + 1) * 128], transpose=True)

    # persistent-ish attention tensors
    qkt_pool = ctx.enter_context(tc.tile_pool(name="qkt", bufs=10))
    v3_pool = ctx.enter_context(tc.tile_pool(name="v3", bufs=1))
    oo_pool = ctx.enter_context(tc.tile_pool(name="oo", bufs=1))
    dn_pool = ctx.enter_context(tc.tile_pool(name="dn", bufs=1))
    ws_pool = ctx.enter_context(tc.tile_pool(name="ws", bufs=18))
    e_pool = ctx.enter_context(tc.tile_pool(name="epool", bufs=6))
    drain_pool = ctx.enter_context(tc.tile_pool(name="drain", bufs=4))

    denom = dn_pool.tile([HPC, S], BF16, tag="denom", name="denom")
    recip = dn_pool.tile([HPC, S], BF16, tag="recip", name="recip")
    oo = [oo_pool.tile([128, S], BF16, tag=f"oo{j}", name=f"oo{j}") for j in range(NP)]

    # ---- V projection: V[s, (h,e)] + bias, plus ones column -> V3 ----
    with tc.tile_pool(name="psv", bufs=2, space="PSUM") as psv:
        v3 = [v3_pool.tile([128, HPC * 65], BF16, tag=f"v3_{st}", name=f"v3_{st}") for st in range(NS)]
        wvt = [ws_pool.tile([128, HPC * DH], BF16, tag="wv", name="wv") for dc in range(NDC)]
        for dc in range(NDC):
            nc.sync.dma_start(wvt[dc][:], wv_d[dc * 128:(dc + 1) * 128, :])
        for st in range(NS):
            ps = psv.tile([128, HPC * DH], F32, tag="psv", name="psv")
            for dc in range(NDC):
                nc.tensor.matmul(
                    ps[:],
                    xt[dc][:, st * 128:(st + 1) * 128],
                    wvt[dc][:],
                    start=(dc == 0), stop=(dc == NDC - 1),
                )
            v_view = v3[st][:].rearrange("p (h e) -> p h e", h=HPC)[:, :, 0:DH]
            nc.vector.tensor_add(
                v_view,
                ps[:].rearrange("p (h e) -> p h e", h=HPC),
                bv[:].rearrange("p (h e) -> p h e", h=HPC),
            )
            nc.vector.memset(
                v3[st][:].rearrange("p (h e) -> p h e", h=HPC)[:, :, DH:65], 1.0)

    # ---- Q^T/K^T projection ----
    qt = [qkt_pool.tile([128, S], BF16, tag="qkt", name="qkt") for j in range(NP)]
    kt = [qkt_pool.tile([128, S], BF16, tag="qkt", name="qkt") for j in range(NP)]

    def emit_qk_group(psum_pool, mt, nb, wt):
        t, j = mt // NP, mt % NP
        dst = qt[j] if t == 0 else kt[j]
        ps = psum_pool.tile([128, 512], F32, tag="st", name="psqk")
        for dc in range(NDC):
            nc.tensor.matmul(
                ps[:], wt[dc][:], xt[dc][:, nb * 512:(nb + 1) * 512],
                start=(dc == 0), stop=(dc == NDC - 1))
        nc.vector.tensor_scalar_add(
            dst[:, nb * 512:(nb + 1) * 512], ps[:], bqk[:, mt:mt + 1])

    def load_wqk(mt):
        wt = [ws_pool.tile([128, 128], BF16, tag="wqk", name="wqk")
              for dc in range(NDC)]
        for dc in range(NDC):
            nc.sync.dma_start(
                wt[dc][:], wqk_d[dc * 128:(dc + 1) * 128,
                                 mt * 128:(mt + 1) * 128])
        return wt

    # pair 0 upfront (attention on heads 0/1 needs it)
    with tc.tile_pool(name="psp", bufs=2, space="PSUM") as psp0:
        for mt in (0, NP):
            wt = load_wqk(mt)
            for nb in range(NQ):
                emit_qk_group(psp0, mt, nb, wt)

    def deferred_qk_groups(pool):
        for j in range(1, NP):
            for t in range(2):
                mt = t * NP + j
                wt = load_wqk(mt)
                for nb in range(NQ):
                    yield lambda mt=mt, nb=nb, wt=wt: emit_qk_group(pool, mt, nb, wt)

    # ---- attention, head by head ----
    with (
        tc.tile_pool(name="pst", bufs=2, space="PSUM") as pst,
        tc.tile_pool(name="pso", bufs=4, space="PSUM") as pso,
    ):
        deferred = deferred_qk_groups(pst)
        kc_iter = 0
        for h in range(HPC):
            j, po = h // 2, (h % 2) * 64
            opsum = [pso.tile([65, 512], F32, tag="opsum", name="opsum") for qb in range(NQ)]
            for kc in range(NS):
                qb0 = kc // 4
                c0 = kc * 128          # first valid column (q >= k)
                e_t = e_pool.tile([128, S], BF16, tag="e", name="e")
                # S^T row-chunk, 512-bank-aligned pieces from c0, exp to E
                seg0 = c0
                while seg0 < S:
                    segw = min(1024 - seg0 % 1024, S - seg0)
                    st_ps = pst.tile([128, 1024], F32, tag="st", name="st")
                    sb = seg0 % 1024
                    p0 = seg0
                    while p0 < seg0 + segw:
                        pw = min(512 - p0 % 512, seg0 + segw - p0)
                        nc.tensor.matmul(
                            st_ps[:, p0 - seg0 + sb:p0 - seg0 + sb + pw],
                            kt[j][po:po + 64, kc * 128:(kc + 1) * 128],
                            qt[j][po:po + 64, p0:p0 + pw],
                            start=True, stop=True,
                        )
                        p0 += pw
                    nc.scalar.activation(
                        e_t[:, seg0:seg0 + segw], st_ps[:, sb:sb + segw],
                        AF.Exp, scale=0.125)
                    seg0 += segw
                if kc_iter % 4 == 3:
                    g = next(deferred, None)
                    if g is not None:
                        g()
                kc_iter += 1
                # causal mask on the diagonal 128x128 block (gpsimd: idle engine)
                nc.gpsimd.tensor_mul(
                    e_t[:, c0:c0 + 128], e_t[:, c0:c0 + 128], tri[:])
                # PV accumulate (diagonal q-block gets partial width)
                for qb in range(qb0, NQ):
                    lo = max(qb * 512, c0)
                    nc.tensor.matmul(
                        opsum[qb][:, lo - qb * 512:512],
                        v3[kc][:, h * 65:(h + 1) * 65],
                        e_t[:, lo:(qb + 1) * 512],
                        start=(kc == 0), stop=(kc == min(NS - 1, 4 * qb + 3)),
                    )
            for qb in range(NQ):
                stg = drain_pool.tile([65, 512], BF16, tag="stg", name="stg")
                nc.vector.tensor_copy(stg[:], opsum[qb][:])
                nc.sync.dma_start(
                    oo[j][po:po + 64, qb * 512:(qb + 1) * 512], stg[0:64, :])
                nc.sync.dma_start(
                    denom[h:h + 1, qb * 512:(qb + 1) * 512], stg[64:65, :])

    # ---- normalize + output projection ----
    with nc.allow_low_precision(reason="softmax denom reciprocal in bf16"):
        nc.vector.reciprocal(recip[:], denom[:])
    with (
        tc.tile_pool(name="psr", bufs=2, space="PSUM") as psr,
        tc.tile_pool(name="psz", bufs=3, space="PSUM") as psz,
    ):
        for j in range(NP):
            for nb in range(NQ):
                rps = psr.tile([128, 512], F32, tag="rps", name="rps")
                nc.tensor.matmul(
                    rps[:],
                    sel[:, j * 128:(j + 1) * 128],
                    recip[:, nb * 512:(nb + 1) * 512],
                    start=True, stop=True,
                )
                nc.vector.tensor_mul(
                    oo[j][:, nb * 512:(nb + 1) * 512],
                    oo[j][:, nb * 512:(nb + 1) * 512], rps[:])
        for mt in range(NS):
            ob = drain_pool.tile([128, D], F32, tag="ob", name="ob")
            for nb in range(2):
                ps = psz.tile([128, 512], F32, tag="psz", name="psz")
                for j in range(NP):
                    nc.tensor.matmul(
                        ps[:],
                        oo[j][:, mt * 128:(mt + 1) * 128],
                        wo[j][:, nb * 512:(nb + 1) * 512],
                        start=(j == 0), stop=(j == NP - 1),
                    )
                nc.vector.tensor_copy(ob[:, nb * 512:(nb + 1) * 512], ps[:])
            nc.sync.dma_start(out_d[mt * 128:(mt + 1) * 128, :], ob[:])


def _build():
    nc = bacc.Bacc("TRN2", target_bir_lowering=False, debug=False)
    x_d = nc.dram_tensor("x_s", [S, D], BF16, kind="ExternalInput").ap()
    wqk_d = nc.dram_tensor("wqk", [D, 2 * HPC * DH], BF16, kind="ExternalInput").ap()
    bqk_d = nc.dram_tensor("bqk", [128, 8], F32, kind="ExternalInput").ap()
    wv_d = nc.dram_tensor("wv", [D, HPC * DH], BF16, kind="ExternalInput").ap()
    bv_d = nc.dram_tensor("bvb", [128, HPC * DH], F32, kind="ExternalInput").ap()
    wo_d = nc.dram_tensor("wo", [HPC * DH, D], BF16, kind="ExternalInput").ap()
    sel_d = nc.dram_tensor("sel", [HPC, NP * 128], BF16, kind="ExternalInput").ap()
    tri_d = nc.dram_tensor("tri", [128, 128], BF16, kind="ExternalInput").ap()
    out_d = nc.dram_tensor("out_s", [S, D], F32, kind="ExternalOutput").ap()
    io = (x_d, wqk_d, bqk_d, wv_d, bv_d, wo_d, sel_d, tri_d, out_d)
    with tile.TileContext(nc) as tc:
        with ExitStack() as ctx:
            _emit(ctx, tc, io)
    nc.compile()
    return nc


_NC = None


def _get_nc():
    global _NC
    if _NC is None:
        _NC = _build()
    return _NC


def _host_inputs(x, w_qkv, b_qkv, w_out):
    """Per-head-group shared weight arrays + per-core x."""
    maps = []
    hg_arrs = []
    for hg in range(2):
        hs = slice(hg * HPC, (hg + 1) * HPC)
        wq = np.asarray(w_qkv[:, 0, hs, :]).reshape(D, HPC * DH)
        wk = np.asarray(w_qkv[:, 1, hs, :]).reshape(D, HPC * DH)
        import ml_dtypes
        wqk = np.concatenate([wq, wk], axis=1).astype(ml_dtypes.bfloat16)
        bq = np.asarray(b_qkv[0, hs, :]).reshape(HPC * DH)
        bk = np.asarray(b_qkv[1, hs, :]).reshape(HPC * DH)
        bqk = np.zeros((128, 8), np.float32)
        for mt in range(8):
            t, j = mt // NP, mt % NP
            src = bq if t == 0 else bk
            bqk[:, mt] = src[j * 128:(j + 1) * 128]
        wv = np.asarray(w_qkv[:, 2, hs, :]).reshape(D, HPC * DH).astype(ml_dtypes.bfloat16)
        bvb = np.broadcast_to(
            np.asarray(b_qkv[2, hs, :]).reshape(1, HPC * DH), (128, HPC * DH)
        ).astype(np.float32)
        wo = np.asarray(w_out[hs]).reshape(HPC * DH, D).astype(ml_dtypes.bfloat16)
        selm = np.zeros((HPC, NP * 128), ml_dtypes.bfloat16)
        for j in range(NP):
            for p in range(128):
                selm[2 * j + p // 64, j * 128 + p] = 1.0
        trim = (np.arange(128)[None, :] >= np.arange(128)[:, None]).astype(
            ml_dtypes.bfloat16)
        hg_arrs.append(dict(wqk=wqk, bqk=bqk, wv=wv, bvb=bvb, wo=wo,
                            sel=selm, tri=trim))
    for c in range(8):
        b, hg = c % B, c // B
        m = dict(hg_arrs[hg])
        import ml_dtypes
        m["x_s"] = np.ascontiguousarray(np.asarray(x[b]).astype(ml_dtypes.bfloat16))
        maps.append(m)
    return maps


def _run(inputs, trace=False, tmpdir=None):
    nc = _get_nc()
    in_maps = _host_inputs(inputs["x"], inputs["w_qkv"], inputs["b_qkv"],
                           inputs["w_out"])
    res = bass_utils.run_bass_kernel_spmd(
        nc, in_maps, core_ids=list(range(8)), trace=trace, tmpdir=tmpdir)
    b_out = np.asarray(inputs["b_out"], dtype=np.float32)
    out = np.empty((B, S, D), np.float32)
    for b in range(B):
        out[b] = (res.results[b]["out_s"] + res.results[b + B]["out_s"]
                  + b_out[None, :])
    return out, res


def kernel(**inputs) -> np.ndarray:
    out, _ = _run(inputs, trace=False)
    return out



# revision 47
# speedup vs baseline: 1.4247x; 1.4247x over previous
"""Causal attention block kernel for TRN2, 8 NeuronCores.

Sharding: 8 cores = 4 batches x 2 head-groups (8 heads each).
Each core computes, for its (batch, head-group):
  qkv = x @ w_qkv + b_qkv ; causal softmax attention ; partial out-proj.
Host sums the two head-group partials per batch and adds b_out.

Per-core layout (q-partition flash attention):
  X^T [d,s] via DMA transpose; Q^T,K^T [64e, 2048s] per head (bf16);
  V augmented [s, (h, 64e + ones)] (bf16).  Per head, per k-chunk kc:
  S^T[k,q] = K^T.T @ Q^T in PSUM, E = exp(S/8) -> SBUF bf16 (wide ACT ops),
  causal mask on the diagonal 128x128 block (gpsimd).  PV runs in
  q-partition layout: for each 128-wide q-tile >= kc,
  O[q,65] += E_kc[:, qtile].T @ V_kc (ones column gives the softmax
  denominator in column 64) accumulated over kc in PSUM.  Normalize with
  reciprocal + per-partition tensor_scalar (cheap: denominator lives on
  the q partition axis), PE-transpose head pairs back to [he, q] for the
  out-projection, which accumulates over head pairs into [q, d] PSUM.
"""

import numpy as np
from contextlib import ExitStack

import concourse.bacc as bacc
import concourse.bass as bass
import concourse.mybir as mybir
import concourse.tile as tile
from concourse import bass_utils

F32 = mybir.dt.float32
BF16 = mybir.dt.bfloat16
AF = mybir.ActivationFunctionType

B, S, D, H, DH = 4, 2048, 1024, 16, 64
DEBUG = False
HPC = 8            # heads per core
NP = 4             # head pairs per core
NS = S // 128      # 16 s-tiles / k-chunks
NQ = S // 512      # 4 q-blocks
NDC = D // 128     # 8 d-chunks



def _emit(ctx: ExitStack, tc: tile.TileContext, io):
    nc = tc.nc
    x_d, wall_d, bqk_d, bvb_d, wo_d, tri_d, id_d, out_d = io[:8]

    const = ctx.enter_context(tc.tile_pool(name="const", bufs=1))

    # ---- resident constants (small ones first) ----
    bqk = const.tile([128, 8], F32, tag="bqk", name="bqk")
    nc.sync.dma_start(bqk[:], bqk_d[:])
    bvb = const.tile([128, HPC * DH], F32, tag="bvb", name="bvb")
    nc.sync.dma_start(bvb[:], bvb_d[:])
    tri = const.tile([128, 128], BF16, tag="tri", name="tri")
    nc.sync.dma_start(tri[:], tri_d[:])
    ident = const.tile([128, 128], BF16, tag="ident", name="ident")
    nc.sync.dma_start(ident[:], id_d[:])

    # X^T [d, s] (host pre-transposed), interleaved with the fused
    # (wq|wk|wv) weight wall so projection chunks unblock early.
    xt = [const.tile([128, S], BF16, tag=f"xt{dc}", name=f"xt{dc}") for dc in range(NDC)]
    wall = [const.tile([128, 1536], BF16, tag=f"wall{dc}", name=f"wall{dc}")
            for dc in range(NDC)]
    for dc in range(NDC):
        nc.sync.dma_start(xt[dc][:], x_d[dc * 128:(dc + 1) * 128, :])
        nc.sync.dma_start(wall[dc][:], wall_d[dc * 128:(dc + 1) * 128, :])
    wo = [const.tile([128, D], BF16, tag=f"wo{j}", name=f"wo{j}") for j in range(NP)]
    for j in range(NP):
        nc.sync.dma_start(wo[j][:], wo_d[j * 128:(j + 1) * 128, :])

    # persistent attention tensors
    qkt_pool = ctx.enter_context(tc.tile_pool(name="qkt", bufs=1))
    v3_pool = ctx.enter_context(tc.tile_pool(name="v3", bufs=1))
    oo_pool = ctx.enter_context(tc.tile_pool(name="oo", bufs=1))
    op_pool = ctx.enter_context(tc.tile_pool(name="opair", bufs=1))
    rc_pool = ctx.enter_context(tc.tile_pool(name="rc", bufs=8))
    # compact triangular E storage: chunk kc holds columns [kc*128, S),
    # resident for the whole head (PV is q-tile-outer).  Early chunks are
    # double-buffered so the next head's scores can run several chunks
    # ahead of this head's last PV q-tiles (pipeline skew).
    e_pool = ctx.enter_context(tc.tile_pool(name="epool", bufs=1))
    e_pool2 = ctx.enter_context(tc.tile_pool(name="epool2", bufs=2))
    em_pool = ctx.enter_context(tc.tile_pool(name="em", bufs=7))
    ob_pool = ctx.enter_context(tc.tile_pool(name="ob", bufs=3))

    qt = [qkt_pool.tile([128, S], BF16, tag=f"qt{j}", name=f"qt{j}") for j in range(NP)]
    kt = [qkt_pool.tile([128, S], BF16, tag=f"kt{j}", name=f"kt{j}") for j in range(NP)]
    v3 = [v3_pool.tile([128, HPC * 65], BF16, tag=f"v3_{st}", name=f"v3_{st}")
          for st in range(NS)]
    oo = [oo_pool.tile([128, S], BF16, tag=f"oo{j}", name=f"oo{j}") for j in range(NP)]
    opair = [op_pool.tile([128, 128], BF16, tag=f"opr{t}", name=f"opr{t}")
             for t in range(NS)]

    # single PSUM footprint for the whole kernel (8 banks):
    #   pst: 2x[128,1024] wide score slots (also recycled by the out-proj)
    #   psm: 2x one-bank slots - narrow late-kc scores, PE transposes,
    #        and drip-fed projection groups (deepens the exp pipeline)
    #   pop: 2 rotating PV accumulator banks (each accumulation group must
    #        own a full bank: start= clears the whole bank)
    pst = ctx.enter_context(tc.tile_pool(name="pst", bufs=2, space="PSUM"))
    psm = ctx.enter_context(tc.tile_pool(name="psm", bufs=2, space="PSUM"))
    pop = ctx.enter_context(tc.tile_pool(name="pop", bufs=2, space="PSUM"))

    # ---- projection group emitters ----
    def emit_v_group(st, pool=None, tag="st"):
        ps = (pool or pst).tile([128, HPC * DH], F32, tag=tag, name="psv")
        for dc in range(NDC):
            nc.tensor.matmul(
                ps[:], xt[dc][:, st * 128:(st + 1) * 128], wall[dc][:, 1024:1536],
                start=(dc == 0), stop=(dc == NDC - 1))
        v_view = v3[st][:].rearrange("p (h e) -> p h e", h=HPC)
        nc.vector.tensor_add(
            v_view[:, :, 0:DH],
            ps[:].rearrange("p (h e) -> p h e", h=HPC),
            bvb[:].rearrange("p (h e) -> p h e", h=HPC))
        nc.vector.memset(v_view[:, :, DH:65], 1.0)

    def emit_qk_group(mt, nb, pool=None, tag="st"):
        t, j = mt // NP, mt % NP
        dst = qt[j] if t == 0 else kt[j]
        ps = (pool or pst).tile([128, 512], F32, tag=tag, name="psqk")
        for dc in range(NDC):
            nc.tensor.matmul(
                ps[:], wall[dc][:, mt * 128:(mt + 1) * 128],
                xt[dc][:, nb * 512:(nb + 1) * 512],
                start=(dc == 0), stop=(dc == NDC - 1))
        nc.vector.tensor_scalar_add(
            dst[:, nb * 512:(nb + 1) * 512], ps[:], bqk[:, mt:mt + 1])

    # upfront: just enough for head 0 to start, spread across all idle
    # psum bank tags (attention hasn't claimed them yet)
    emit_qk_group(4, 0, tag="st")
    emit_qk_group(0, 0, tag="st")
    emit_qk_group(0, 1, pool=psm, tag="sm")
    emit_qk_group(0, 2, pool=psm, tag="sm")
    emit_qk_group(0, 3, pool=pop, tag="pv")
    emit_v_group(0, pool=pop, tag="pv")
    emit_v_group(1, tag="st")

    # everything else drip-feeds into the attention instruction stream,
    # paced so each projection lands just before its deadline (kt/qt pair p
    # before head 2p; v3[st] early in head 0) and fills PE idle in the
    # otherwise exp-paced middle heads.  Injected groups use the "tp" bank
    # so both score slots stay free to run the exp pipeline ahead.
    deferred = [("qk", 4, 1), ("v", 2, 0), ("qk", 4, 2), ("v", 3, 0),
                ("qk", 4, 3), ("v", 4, 0)]
    deferred += [("v", st, 0) for st in range(5, NS)]
    # per pair jj (heads 2jj/2jj+1 start at iter 32*jj): qt groups must all
    # land just before the pair's first head; kt columns stream in with kc.
    for jj in (1, 2, 3):
        deferred += [("qk", jj, nb) for nb in range(NQ)]
        deferred += [("qk", NP + jj, nb) for nb in range(NQ)]
    deferred.reverse()  # pop() from the end
    # injection slots: iter -> count (deadline-aware, as late as legal so PE
    # filler lands inside the exp-paced attention middle)
    _sched = {i: 2 for i in range(1, 9)}
    _sched[9] = 1
    for base in (32, 64, 96):              # pair jj = base//32 starts at base
        for i in (-14, -11, -8, -5):       # qt groups: all before the pair
            _sched[base + i] = 1
        for i in (-4, 0, 4, 8):            # kt group nb=k needed by base+4k
            _sched[base + i] = 1

    def inject(n):
        for _ in range(n):
            if not deferred:
                return
            kind, a, b = deferred.pop()
            if kind == "qk":
                emit_qk_group(a, b, pool=psm, tag="sm")
            else:
                emit_v_group(a, pool=psm, tag="sm")

    def emit_scores(h, kc, e_of, em_of):
        """S^T chunk -> exp -> compact e (cols kc*128..S) + masked diag em."""
        j, po = h // 2, (h % 2) * 64
        c0 = kc * 128
        pool = e_pool2 if kc < 4 else e_pool
        e_t = pool.tile([128, S - c0], BF16, tag=f"e{kc}", name=f"e{kc}")
        seg0 = c0
        while seg0 < S:
            segw = min(1024 - seg0 % 1024, S - seg0)
            if kc >= 12:
                st_ps = psm.tile([128, 512], F32, tag="sm", name="sm")
            else:
                st_ps = pst.tile([128, 1024], F32, tag="st", name="st")
            sb = seg0 % 1024 if kc < 12 else seg0 % 512
            p0 = seg0
            while p0 < seg0 + segw:
                pw = min(512 - p0 % 512, seg0 + segw - p0)
                nc.tensor.matmul(
                    st_ps[:, p0 - seg0 + sb:p0 - seg0 + sb + pw],
                    kt[j][po:po + 64, kc * 128:(kc + 1) * 128],
                    qt[j][po:po + 64, p0:p0 + pw],
                    start=True, stop=True)
                p0 += pw
            nc.scalar.activation(
                e_t[:, seg0 - c0:seg0 - c0 + segw], st_ps[:, sb:sb + segw],
                AF.Exp, scale=0.125)
            seg0 += segw
        # causal mask of the diagonal 128x128 block into a separate tile
        # (gpsimd: idle engine) so off-diagonal PV matmuls don't wait on it
        em = em_pool.tile([128, 128], BF16, tag="em", name="em")
        nc.vector.tensor_mul(em[:], e_t[:, 0:128], tri[:])
        if DEBUG and h == 0 and kc == 0:
            nc.sync.dma_start(io[-1][4][0:128, :], e_t[:])
            nc.sync.dma_start(io[-1][4][128:256, 0:128], em[:])
        e_of[kc] = e_t
        em_of[kc] = em

    def emit_outproj(mts):
        for mt in mts:
            ps = pst.tile([128, D], F32, tag="st", name="psz")
            for nb in range(2):
                for jj in range(NP):
                    nc.tensor.matmul(
                        ps[:, nb * 512:(nb + 1) * 512],
                        oo[jj][:, mt * 128:(mt + 1) * 128],
                        wo[jj][:, nb * 512:(nb + 1) * 512],
                        start=(jj == 0), stop=(jj == NP - 1))
            ob = ob_pool.tile([128, D], BF16, tag="ob", name="ob")
            nc.scalar.copy(ob[:], ps[:])
            nc.sync.dma_start(out_d[mt * 128:(mt + 1) * 128, :], ob[:])

    def emit_pv(h, t, e_of, em_of):
        """PV for q-tile t: one short-lived accumulation group per bank,
        then normalize (and transpose once the head pair is complete)."""
        j, po = h // 2, (h % 2) * 64
        ps = pop.tile([128, 65], F32, tag="pv", name="pv")
        for kc in range(t + 1):
            lhsT = (em_of[kc][:] if kc == t
                    else e_of[kc][:, (t - kc) * 128:(t - kc + 1) * 128])
            nc.tensor.matmul(
                ps[:], lhsT, v3[kc][:, h * 65:(h + 1) * 65],
                start=(kc == 0), stop=(kc == t))
        r = rc_pool.tile([128, 1], F32, tag="rc", name="rc")
        nc.vector.reciprocal(r[:], ps[:, 64:65])
        nc.vector.tensor_scalar_mul(
            opair[t][:, po:po + 64], ps[:, 0:DH], r[:])
        if po:  # pair complete: transpose back to [he, q] for out-proj
            tps = psm.tile([128, 128], BF16, tag="sm", name="tps")
            nc.tensor.transpose(tps[:], opair[t][:], ident[:])
            if h == HPC - 1:  # ACT is idle in the tail; DVE is not
                nc.scalar.copy(oo[j][:, t * 128:(t + 1) * 128], tps[:])
                emit_outproj([t])  # all pairs done: out-proj for this q-tile
            else:
                nc.vector.tensor_copy(oo[j][:, t * 128:(t + 1) * 128], tps[:])

    # ---- attention, software-pipelined four k-chunks ahead ----
    from collections import deque
    pending = deque()
    kc_iter = 0
    e_of, em_of = [None] * NS, [None] * NS
    for h in range(HPC):
        for kc in range(NS):
            inject(_sched.get(kc_iter, 0))
            kc_iter += 1
            emit_scores(h, kc, e_of, em_of)
            pending.append((h, kc, list(e_of), list(em_of)))
            if len(pending) > 4:
                emit_pv(*pending.popleft())
    while pending:
        emit_pv(*pending.popleft())

    if DEBUG:
        dbg_qt, dbg_kt, dbg_v3, dbg_oo, dbg_e, dbg_op = io[-1]
        for t in range(NS):
            nc.sync.dma_start(dbg_op[t * 128:(t + 1) * 128, :], opair[t][:])
        for jj in range(NP):
            nc.sync.dma_start(dbg_qt[jj * 128:(jj + 1) * 128, :], qt[jj][:])
            nc.sync.dma_start(dbg_kt[jj * 128:(jj + 1) * 128, :], kt[jj][:])
            nc.sync.dma_start(dbg_oo[jj * 128:(jj + 1) * 128, :], oo[jj][:])
        for st in range(NS):
            nc.sync.dma_start(dbg_v3[st * 128:(st + 1) * 128, :], v3[st][:])


def _build():
    nc = bacc.Bacc("TRN2", target_bir_lowering=False, debug=False)
    x_d = nc.dram_tensor("x_s", [D, S], BF16, kind="ExternalInput").ap()
    wall_d = nc.dram_tensor("wall", [D, 1536], BF16, kind="ExternalInput").ap()
    bqk_d = nc.dram_tensor("bqk", [128, 8], F32, kind="ExternalInput").ap()
    bvb_d = nc.dram_tensor("bvb", [128, HPC * DH], F32, kind="ExternalInput").ap()
    wo_d = nc.dram_tensor("wo", [HPC * DH, D], BF16, kind="ExternalInput").ap()
    tri_d = nc.dram_tensor("tri", [128, 128], BF16, kind="ExternalInput").ap()
    id_d = nc.dram_tensor("ident", [128, 128], BF16, kind="ExternalInput").ap()
    out_d = nc.dram_tensor("out_s", [S, D], BF16, kind="ExternalOutput").ap()
    io = [x_d, wall_d, bqk_d, bvb_d, wo_d, tri_d, id_d, out_d]
    if DEBUG:
        dbg = (nc.dram_tensor("dbg_qt", [512, S], BF16, kind="ExternalOutput").ap(),
               nc.dram_tensor("dbg_kt", [512, S], BF16, kind="ExternalOutput").ap(),
               nc.dram_tensor("dbg_v3", [NS * 128, HPC * 65], BF16, kind="ExternalOutput").ap(),
               nc.dram_tensor("dbg_oo", [512, S], BF16, kind="ExternalOutput").ap(),
               nc.dram_tensor("dbg_e", [256, S], BF16, kind="ExternalOutput").ap(),
               nc.dram_tensor("dbg_op", [NS * 128, 128], BF16, kind="ExternalOutput").ap())
        io.append(dbg)
    with tile.TileContext(nc) as tc:
        with ExitStack() as ctx:
            _emit(ctx, tc, io)
    nc.compile()
    return nc


_NC = None


def _get_nc():
    global _NC
    if _NC is None:
        _NC = _build()
    return _NC


def _host_inputs(x, w_qkv, b_qkv, w_out):
    """Per-head-group shared weight arrays + per-core x."""
    import ml_dtypes
    maps = []
    hg_arrs = []
    for hg in range(2):
        hs = slice(hg * HPC, (hg + 1) * HPC)
        wq = np.asarray(w_qkv[:, 0, hs, :]).reshape(D, HPC * DH)
        wk = np.asarray(w_qkv[:, 1, hs, :]).reshape(D, HPC * DH)
        wv = np.asarray(w_qkv[:, 2, hs, :]).reshape(D, HPC * DH)
        wall = np.concatenate([wq, wk, wv], axis=1).astype(ml_dtypes.bfloat16)
        bq = np.asarray(b_qkv[0, hs, :]).reshape(HPC * DH)
        bk = np.asarray(b_qkv[1, hs, :]).reshape(HPC * DH)
        bqk = np.zeros((128, 8), np.float32)
        for mt in range(8):
            t, j = mt // NP, mt % NP
            src = bq if t == 0 else bk
            bqk[:, mt] = src[j * 128:(j + 1) * 128]
        bvb = np.broadcast_to(
            np.asarray(b_qkv[2, hs, :]).reshape(1, HPC * DH), (128, HPC * DH)
        ).astype(np.float32)
        wo = np.asarray(w_out[hs]).reshape(HPC * DH, D).astype(ml_dtypes.bfloat16)
        trim = (np.arange(128)[None, :] >= np.arange(128)[:, None]).astype(
            ml_dtypes.bfloat16)
        idm = np.eye(128, dtype=ml_dtypes.bfloat16)
        hg_arrs.append(dict(wall=wall, bqk=bqk, bvb=bvb, wo=wo,
                            tri=trim, ident=idm))
    for c in range(8):
        b, hg = c % B, c // B
        m = dict(hg_arrs[hg])
        m["x_s"] = np.ascontiguousarray(
            np.asarray(x[b]).astype(ml_dtypes.bfloat16).T)
        maps.append(m)
    return maps


def _run(inputs, trace=False, tmpdir=None):
    nc = _get_nc()
    in_maps = _host_inputs(inputs["x"], inputs["w_qkv"], inputs["b_qkv"],
                           inputs["w_out"])
    res = bass_utils.run_bass_kernel_spmd(
        nc, in_maps, core_ids=list(range(8)), trace=trace, tmpdir=tmpdir)
    b_out = np.asarray(inputs["b_out"], dtype=np.float32)
    out = np.empty((B, S, D), np.float32)
    for b in range(B):
        out[b] = (res.results[b]["out_s"].astype(np.float32)
                  + res.results[b + B]["out_s"].astype(np.float32)
                  + b_out[None, :])
    return out, res


def kernel(**inputs) -> np.ndarray:
    out, _ = _run(inputs, trace=False)
    return out


# revision 55
# speedup vs baseline: 1.4374x; 1.0089x over previous
"""Causal attention block kernel for TRN2, 8 NeuronCores.

Sharding: 8 cores = 4 batches x 2 head-groups (8 heads each).
Each core computes, for its (batch, head-group):
  qkv = x @ w_qkv + b_qkv ; causal softmax attention ; partial out-proj.
Host sums the two head-group partials per batch and adds b_out.

Per-core layout (q-partition flash attention):
  X^T [d,s] via DMA transpose; Q^T,K^T [64e, 2048s] per head (bf16);
  V augmented [s, (h, 64e + ones)] (bf16).  Per head, per k-chunk kc:
  S^T[k,q] = K^T.T @ Q^T in PSUM, E = exp(S/8) -> SBUF bf16 (wide ACT ops),
  causal mask on the diagonal 128x128 block (gpsimd).  PV runs in
  q-partition layout: for each 128-wide q-tile >= kc,
  O[q,65] += E_kc[:, qtile].T @ V_kc (ones column gives the softmax
  denominator in column 64) accumulated over kc in PSUM.  Normalize with
  reciprocal + per-partition tensor_scalar (cheap: denominator lives on
  the q partition axis), PE-transpose head pairs back to [he, q] for the
  out-projection, which accumulates over head pairs into [q, d] PSUM.
"""

import numpy as np
from contextlib import ExitStack

import concourse.bacc as bacc
import concourse.bass as bass
import concourse.mybir as mybir
import concourse.tile as tile
from concourse import bass_utils

F32 = mybir.dt.float32
BF16 = mybir.dt.bfloat16
AF = mybir.ActivationFunctionType

B, S, D, H, DH = 4, 2048, 1024, 16, 64
DEBUG = False
HPC = 8            # heads per core
NP = 4             # head pairs per core
NS = S // 128      # 16 s-tiles / k-chunks
NQ = S // 512      # 4 q-blocks
NDC = D // 128     # 8 d-chunks



def _emit(ctx: ExitStack, tc: tile.TileContext, io):
    nc = tc.nc
    x_d, wall_d, bqk_d, bvb_d, wo_d, tri_d, id_d, out_d = io[:8]

    const = ctx.enter_context(tc.tile_pool(name="const", bufs=1))

    # ---- resident constants (small ones first) ----
    bqk = const.tile([128, 8], F32, tag="bqk", name="bqk")
    nc.sync.dma_start(bqk[:], bqk_d[:])
    bvb = const.tile([128, HPC * DH], F32, tag="bvb", name="bvb")
    nc.sync.dma_start(bvb[:], bvb_d[:])
    tri = const.tile([128, 128], BF16, tag="tri", name="tri")
    nc.sync.dma_start(tri[:], tri_d[:])
    ident = const.tile([128, 128], BF16, tag="ident", name="ident")
    nc.sync.dma_start(ident[:], id_d[:])

    # X^T [d, s] (host pre-transposed), interleaved with the fused
    # (wq|wk|wv) weight wall so projection chunks unblock early.
    xt = [const.tile([128, S], BF16, tag=f"xt{dc}", name=f"xt{dc}") for dc in range(NDC)]
    wall = [const.tile([128, 1536], BF16, tag=f"wall{dc}", name=f"wall{dc}")
            for dc in range(NDC)]
    for dc in range(NDC):
        nc.sync.dma_start(xt[dc][:], x_d[dc * 128:(dc + 1) * 128, :])
        nc.sync.dma_start(wall[dc][:], wall_d[dc * 128:(dc + 1) * 128, :])
    wo = [const.tile([128, D], BF16, tag=f"wo{j}", name=f"wo{j}") for j in range(NP)]
    for j in range(NP):
        nc.sync.dma_start(wo[j][:], wo_d[j * 128:(j + 1) * 128, :])

    # persistent attention tensors
    qkt_pool = ctx.enter_context(tc.tile_pool(name="qkt", bufs=1))
    v3_pool = ctx.enter_context(tc.tile_pool(name="v3", bufs=1))
    oo_pool = ctx.enter_context(tc.tile_pool(name="oo", bufs=1))
    op_pool = ctx.enter_context(tc.tile_pool(name="opair", bufs=1))
    rc_pool = ctx.enter_context(tc.tile_pool(name="rc", bufs=8))
    # compact triangular E storage: chunk kc holds columns [kc*128, S),
    # resident for the whole head (PV is q-tile-outer).  Early chunks are
    # double-buffered so the next head's scores can run several chunks
    # ahead of this head's last PV q-tiles (pipeline skew).
    e_pool = ctx.enter_context(tc.tile_pool(name="epool", bufs=1))
    e_pool2 = ctx.enter_context(tc.tile_pool(name="epool2", bufs=2))
    em_pool = ctx.enter_context(tc.tile_pool(name="em", bufs=9))
    ob_pool = ctx.enter_context(tc.tile_pool(name="ob", bufs=3))

    qt = [qkt_pool.tile([128, S], BF16, tag=f"qt{j}", name=f"qt{j}") for j in range(NP)]
    kt = [qkt_pool.tile([128, S], BF16, tag=f"kt{j}", name=f"kt{j}") for j in range(NP)]
    v3 = [v3_pool.tile([128, HPC * 65], BF16, tag=f"v3_{st}", name=f"v3_{st}")
          for st in range(NS)]
    oo = [oo_pool.tile([128, S], BF16, tag=f"oo{j}", name=f"oo{j}") for j in range(NP)]
    opair = [op_pool.tile([128, 128], BF16, tag=f"opr{t}", name=f"opr{t}")
             for t in range(NS)]

    # single PSUM footprint for the whole kernel (8 banks):
    #   pst: 2x[128,1024] wide score slots (also recycled by the out-proj)
    #   psm: 2x one-bank slots - narrow late-kc scores, PE transposes,
    #        and drip-fed projection groups (deepens the exp pipeline)
    #   pop: 2 rotating PV accumulator banks (each accumulation group must
    #        own a full bank: start= clears the whole bank)
    pst = ctx.enter_context(tc.tile_pool(name="pst", bufs=2, space="PSUM"))
    psm = ctx.enter_context(tc.tile_pool(name="psm", bufs=2, space="PSUM"))
    pop = ctx.enter_context(tc.tile_pool(name="pop", bufs=2, space="PSUM"))

    # ---- projection group emitters ----
    def emit_v_group(st, pool=None, tag="st"):
        ps = (pool or pst).tile([128, HPC * DH], F32, tag=tag, name="psv")
        for dc in range(NDC):
            nc.tensor.matmul(
                ps[:], xt[dc][:, st * 128:(st + 1) * 128], wall[dc][:, 1024:1536],
                start=(dc == 0), stop=(dc == NDC - 1))
        v_view = v3[st][:].rearrange("p (h e) -> p h e", h=HPC)
        nc.vector.tensor_add(
            v_view[:, :, 0:DH],
            ps[:].rearrange("p (h e) -> p h e", h=HPC),
            bvb[:].rearrange("p (h e) -> p h e", h=HPC))
        nc.vector.memset(v_view[:, :, DH:65], 1.0)

    def emit_qk_group(mt, nb, pool=None, tag="st"):
        t, j = mt // NP, mt % NP
        dst = qt[j] if t == 0 else kt[j]
        ps = (pool or pst).tile([128, 512], F32, tag=tag, name="psqk")
        for dc in range(NDC):
            nc.tensor.matmul(
                ps[:], wall[dc][:, mt * 128:(mt + 1) * 128],
                xt[dc][:, nb * 512:(nb + 1) * 512],
                start=(dc == 0), stop=(dc == NDC - 1))
        nc.vector.tensor_scalar_add(
            dst[:, nb * 512:(nb + 1) * 512], ps[:], bqk[:, mt:mt + 1])

    # upfront: just enough for head 0 to start, spread across all idle
    # psum bank tags (attention hasn't claimed them yet)
    emit_qk_group(4, 0, tag="st")
    emit_qk_group(0, 0, tag="st")
    emit_qk_group(0, 1, pool=psm, tag="sm")
    emit_qk_group(0, 2, pool=psm, tag="sm")
    emit_qk_group(0, 3, pool=pop, tag="pv")
    emit_v_group(0, pool=pop, tag="pv")
    emit_v_group(1, tag="st")

    # everything else drip-feeds into the attention instruction stream,
    # paced so each projection lands just before its deadline (kt/qt pair p
    # before head 2p; v3[st] early in head 0) and fills PE idle in the
    # otherwise exp-paced middle heads.  Injected groups use the "tp" bank
    # so both score slots stay free to run the exp pipeline ahead.
    deferred = [("qk", 4, 1), ("v", 2, 0), ("qk", 4, 2), ("v", 3, 0),
                ("qk", 4, 3), ("v", 4, 0)]
    deferred += [("v", st, 0) for st in range(5, NS)]
    # per pair jj (heads 2jj/2jj+1 start at iter 32*jj): qt groups must all
    # land just before the pair's first head; kt columns stream in with kc.
    for jj in (1, 2, 3):
        deferred += [("qk", jj, nb) for nb in range(NQ)]
        deferred += [("qk", NP + jj, nb) for nb in range(NQ)]
    deferred.reverse()  # pop() from the end
    # injection slots: iter -> count (deadline-aware, as late as legal so PE
    # filler lands inside the exp-paced attention middle)
    _sched = {i: 2 for i in range(1, 9)}
    _sched[9] = 1
    for base in (32, 64, 96):              # pair jj = base//32 starts at base
        for i in (-14, -11, -8, -5):       # qt groups: all before the pair
            _sched[base + i] = 1
        for i in (-4, 0, 4, 8):            # kt group nb=k needed by base+4k
            _sched[base + i] = 1

    def inject(n):
        for _ in range(n):
            if not deferred:
                return
            kind, a, b = deferred.pop()
            if kind == "qk":
                emit_qk_group(a, b, pool=psm, tag="sm")
            else:
                emit_v_group(a, pool=psm, tag="sm")

    def emit_scores(h, kc, e_of, em_of):
        """S^T chunk -> exp -> compact e (cols kc*128..S) + masked diag em."""
        j, po = h // 2, (h % 2) * 64
        c0 = kc * 128
        pool = e_pool2 if kc < 6 else e_pool
        e_t = pool.tile([128, S - c0], BF16, tag=f"e{kc}", name=f"e{kc}")
        seg0 = c0
        while seg0 < S:
            segw = min(1024 - seg0 % 1024, S - seg0)
            if kc >= 12:
                st_ps = psm.tile([128, 512], F32, tag="sm", name="sm")
            else:
                st_ps = pst.tile([128, 1024], F32, tag="st", name="st")
            sb = seg0 % 1024 if kc < 12 else seg0 % 512
            p0 = seg0
            while p0 < seg0 + segw:
                pw = min(512 - p0 % 512, seg0 + segw - p0)
                nc.tensor.matmul(
                    st_ps[:, p0 - seg0 + sb:p0 - seg0 + sb + pw],
                    kt[j][po:po + 64, kc * 128:(kc + 1) * 128],
                    qt[j][po:po + 64, p0:p0 + pw],
                    start=True, stop=True)
                p0 += pw
            nc.scalar.activation(
                e_t[:, seg0 - c0:seg0 - c0 + segw], st_ps[:, sb:sb + segw],
                AF.Exp, scale=0.125)
            seg0 += segw
        # causal mask of the diagonal 128x128 block into a separate tile
        # (gpsimd: idle engine) so off-diagonal PV matmuls don't wait on it
        em = em_pool.tile([128, 128], BF16, tag="em", name="em")
        nc.vector.tensor_mul(em[:], e_t[:, 0:128], tri[:])
        if DEBUG and h == 0 and kc == 0:
            nc.sync.dma_start(io[-1][4][0:128, :], e_t[:])
            nc.sync.dma_start(io[-1][4][128:256, 0:128], em[:])
        e_of[kc] = e_t
        em_of[kc] = em

    def emit_outproj(mts):
        for mt in mts:
            ps = pst.tile([128, D], F32, tag="st", name="psz")
            for nb in range(2):
                for jj in range(NP):
                    nc.tensor.matmul(
                        ps[:, nb * 512:(nb + 1) * 512],
                        oo[jj][:, mt * 128:(mt + 1) * 128],
                        wo[jj][:, nb * 512:(nb + 1) * 512],
                        start=(jj == 0), stop=(jj == NP - 1))
            ob = ob_pool.tile([128, D], BF16, tag="ob", name="ob")
            nc.scalar.copy(ob[:], ps[:])
            nc.sync.dma_start(out_d[mt * 128:(mt + 1) * 128, :], ob[:])

    def emit_pv(h, t, e_of, em_of):
        """PV for q-tile t: one short-lived accumulation group per bank,
        then normalize (and transpose once the head pair is complete)."""
        j, po = h // 2, (h % 2) * 64
        ps = pop.tile([128, 65], F32, tag="pv", name="pv")
        for kc in range(t + 1):
            lhsT = (em_of[kc][:] if kc == t
                    else e_of[kc][:, (t - kc) * 128:(t - kc + 1) * 128])
            nc.tensor.matmul(
                ps[:], lhsT, v3[kc][:, h * 65:(h + 1) * 65],
                start=(kc == 0), stop=(kc == t))
        r = rc_pool.tile([128, 1], F32, tag="rc", name="rc")
        nc.vector.reciprocal(r[:], ps[:, 64:65])
        nc.vector.tensor_scalar_mul(
            opair[t][:, po:po + 64], ps[:, 0:DH], r[:])
        if po:  # pair complete: transpose back to [he, q] for out-proj
            tps = psm.tile([128, 128], BF16, tag="sm", name="tps")
            nc.tensor.transpose(tps[:], opair[t][:], ident[:])
            if h == HPC - 1:  # ACT is idle in the tail; DVE is not
                nc.scalar.copy(oo[j][:, t * 128:(t + 1) * 128], tps[:])
                emit_outproj([t])  # all pairs done: out-proj for this q-tile
            else:
                nc.vector.tensor_copy(oo[j][:, t * 128:(t + 1) * 128], tps[:])

    # ---- attention, software-pipelined four k-chunks ahead ----
    from collections import deque
    pending = deque()
    kc_iter = 0
    e_of, em_of = [None] * NS, [None] * NS
    for h in range(HPC):
        for kc in range(NS):
            inject(_sched.get(kc_iter, 0))
            kc_iter += 1
            emit_scores(h, kc, e_of, em_of)
            pending.append((h, kc, list(e_of), list(em_of)))
            lim = 3 if h == HPC - 1 else 6
            while len(pending) > lim:
                emit_pv(*pending.popleft())
    while pending:
        emit_pv(*pending.popleft())

    if DEBUG:
        dbg_qt, dbg_kt, dbg_v3, dbg_oo, dbg_e, dbg_op = io[-1]
        for t in range(NS):
            nc.sync.dma_start(dbg_op[t * 128:(t + 1) * 128, :], opair[t][:])
        for jj in range(NP):
            nc.sync.dma_start(dbg_qt[jj * 128:(jj + 1) * 128, :], qt[jj][:])
            nc.sync.dma_start(dbg_kt[jj * 128:(jj + 1) * 128, :], kt[jj][:])
            nc.sync.dma_start(dbg_oo[jj * 128:(jj + 1) * 128, :], oo[jj][:])
        for st in range(NS):
            nc.sync.dma_start(dbg_v3[st * 128:(st + 1) * 128, :], v3[st][:])


def _build():
    nc = bacc.Bacc("TRN2", target_bir_lowering=False, debug=False)
    x_d = nc.dram_tensor("x_s", [D, S], BF16, kind="ExternalInput").ap()
    wall_d = nc.dram_tensor("wall", [D, 1536], BF16, kind="ExternalInput").ap()
    bqk_d = nc.dram_tensor("bqk", [128, 8], F32, kind="ExternalInput").ap()
    bvb_d = nc.dram_tensor("bvb", [128, HPC * DH], F32, kind="ExternalInput").ap()
    wo_d = nc.dram_tensor("wo", [HPC * DH, D], BF16, kind="ExternalInput").ap()
    tri_d = nc.dram_tensor("tri", [128, 128], BF16, kind="ExternalInput").ap()
    id_d = nc.dram_tensor("ident", [128, 128], BF16, kind="ExternalInput").ap()
    out_d = nc.dram_tensor("out_s", [S, D], BF16, kind="ExternalOutput").ap()
    io = [x_d, wall_d, bqk_d, bvb_d, wo_d, tri_d, id_d, out_d]
    if DEBUG:
        dbg = (nc.dram_tensor("dbg_qt", [512, S], BF16, kind="ExternalOutput").ap(),
               nc.dram_tensor("dbg_kt", [512, S], BF16, kind="ExternalOutput").ap(),
               nc.dram_tensor("dbg_v3", [NS * 128, HPC * 65], BF16, kind="ExternalOutput").ap(),
               nc.dram_tensor("dbg_oo", [512, S], BF16, kind="ExternalOutput").ap(),
               nc.dram_tensor("dbg_e", [256, S], BF16, kind="ExternalOutput").ap(),
               nc.dram_tensor("dbg_op", [NS * 128, 128], BF16, kind="ExternalOutput").ap())
        io.append(dbg)
    with tile.TileContext(nc) as tc:
        with ExitStack() as ctx:
            _emit(ctx, tc, io)
    nc.compile()
    return nc


_NC = None


def _get_nc():
    global _NC
    if _NC is None:
        _NC = _build()
    return _NC


def _host_inputs(x, w_qkv, b_qkv, w_out):
    """Per-head-group shared weight arrays + per-core x."""
    import ml_dtypes
    maps = []
    hg_arrs = []
    for hg in range(2):
        hs = slice(hg * HPC, (hg + 1) * HPC)
        wq = np.asarray(w_qkv[:, 0, hs, :]).reshape(D, HPC * DH)
        wk = np.asarray(w_qkv[:, 1, hs, :]).reshape(D, HPC * DH)
        wv = np.asarray(w_qkv[:, 2, hs, :]).reshape(D, HPC * DH)
        wall = np.concatenate([wq, wk, wv], axis=1).astype(ml_dtypes.bfloat16)
        bq = np.asarray(b_qkv[0, hs, :]).reshape(HPC * DH)
        bk = np.asarray(b_qkv[1, hs, :]).reshape(HPC * DH)
        bqk = np.zeros((128, 8), np.float32)
        for mt in range(8):
            t, j = mt // NP, mt % NP
            src = bq if t == 0 else bk
            bqk[:, mt] = src[j * 128:(j + 1) * 128]
        bvb = np.broadcast_to(
            np.asarray(b_qkv[2, hs, :]).reshape(1, HPC * DH), (128, HPC * DH)
        ).astype(np.float32)
        wo = np.asarray(w_out[hs]).reshape(HPC * DH, D).astype(ml_dtypes.bfloat16)
        trim = (np.arange(128)[None, :] >= np.arange(128)[:, None]).astype(
            ml_dtypes.bfloat16)
        idm = np.eye(128, dtype=ml_dtypes.bfloat16)
        hg_arrs.append(dict(wall=wall, bqk=bqk, bvb=bvb, wo=wo,
                            tri=trim, ident=idm))
    for c in range(8):
        b, hg = c % B, c // B
        m = dict(hg_arrs[hg])
        m["x_s"] = np.ascontiguousarray(
            np.asarray(x[b]).astype(ml_dtypes.bfloat16).T)
        maps.append(m)
    return maps


def _run(inputs, trace=False, tmpdir=None):
    nc = _get_nc()
    in_maps = _host_inputs(inputs["x"], inputs["w_qkv"], inputs["b_qkv"],
                           inputs["w_out"])
    res = bass_utils.run_bass_kernel_spmd(
        nc, in_maps, core_ids=list(range(8)), trace=trace, tmpdir=tmpdir)
    b_out = np.asarray(inputs["b_out"], dtype=np.float32)
    out = np.empty((B, S, D), np.float32)
    for b in range(B):
        out[b] = (res.results[b]["out_s"].astype(np.float32)
                  + res.results[b + B]["out_s"].astype(np.float32)
                  + b_out[None, :])
    return out, res


def kernel(**inputs) -> np.ndarray:
    out, _ = _run(inputs, trace=False)
    return out


# revision 61
# speedup vs baseline: 1.4439x; 1.0045x over previous
"""Causal attention block kernel for TRN2, 8 NeuronCores.

Sharding: 8 cores = 4 batches x 2 head-groups (8 heads each).
Each core computes, for its (batch, head-group):
  qkv = x @ w_qkv + b_qkv ; causal softmax attention ; partial out-proj.
Host sums the two head-group partials per batch and adds b_out.

Per-core layout (q-partition flash attention):
  X^T [d,s] via DMA transpose; Q^T,K^T [64e, 2048s] per head (bf16);
  V augmented [s, (h, 64e + ones)] (bf16).  Per head, per k-chunk kc:
  S^T[k,q] = K^T.T @ Q^T in PSUM, E = exp(S/8) -> SBUF bf16 (wide ACT ops),
  causal mask on the diagonal 128x128 block (gpsimd).  PV runs in
  q-partition layout: for each 128-wide q-tile >= kc,
  O[q,65] += E_kc[:, qtile].T @ V_kc (ones column gives the softmax
  denominator in column 64) accumulated over kc in PSUM.  Normalize with
  reciprocal + per-partition tensor_scalar (cheap: denominator lives on
  the q partition axis), PE-transpose head pairs back to [he, q] for the
  out-projection, which accumulates over head pairs into [q, d] PSUM.
"""

import numpy as np
from contextlib import ExitStack

import concourse.bacc as bacc
import concourse.bass as bass
import concourse.mybir as mybir
import concourse.tile as tile
from concourse import bass_utils

F32 = mybir.dt.float32
BF16 = mybir.dt.bfloat16
AF = mybir.ActivationFunctionType

B, S, D, H, DH = 4, 2048, 1024, 16, 64
DEBUG = False
HPC = 8            # heads per core
NP = 4             # head pairs per core
NS = S // 128      # 16 s-tiles / k-chunks
NQ = S // 512      # 4 q-blocks
NDC = D // 128     # 8 d-chunks



def _emit(ctx: ExitStack, tc: tile.TileContext, io):
    nc = tc.nc
    x_d, wall_d, bqk_d, bvb_d, wo_d, tri_d, id_d, out_d = io[:8]

    const = ctx.enter_context(tc.tile_pool(name="const", bufs=1))

    # ---- resident constants (small ones first) ----
    bqk = const.tile([128, 8], F32, tag="bqk", name="bqk")
    nc.sync.dma_start(bqk[:], bqk_d[:])
    bvb = const.tile([128, HPC * DH], F32, tag="bvb", name="bvb")
    nc.sync.dma_start(bvb[:], bvb_d[:])
    tri = const.tile([128, 128], BF16, tag="tri", name="tri")
    nc.sync.dma_start(tri[:], tri_d[:])
    ident = const.tile([128, 128], BF16, tag="ident", name="ident")
    nc.sync.dma_start(ident[:], id_d[:])

    # X^T [d, s] (host pre-transposed), interleaved with the fused
    # (wq|wk|wv) weight wall so projection chunks unblock early.
    xt = [const.tile([128, S], BF16, tag=f"xt{dc}", name=f"xt{dc}") for dc in range(NDC)]
    wall = [const.tile([128, 1536], BF16, tag=f"wall{dc}", name=f"wall{dc}")
            for dc in range(NDC)]
    for dc in range(NDC):
        nc.sync.dma_start(xt[dc][:], x_d[dc * 128:(dc + 1) * 128, :])
        nc.sync.dma_start(wall[dc][:], wall_d[dc * 128:(dc + 1) * 128, :])
    wo = [const.tile([128, D], BF16, tag=f"wo{j}", name=f"wo{j}") for j in range(NP)]
    for j in range(NP):
        nc.sync.dma_start(wo[j][:], wo_d[j * 128:(j + 1) * 128, :])

    # persistent attention tensors
    qkt_pool = ctx.enter_context(tc.tile_pool(name="qkt", bufs=1))
    v3_pool = ctx.enter_context(tc.tile_pool(name="v3", bufs=1))
    oo_pool = ctx.enter_context(tc.tile_pool(name="oo", bufs=1))
    op_pool = ctx.enter_context(tc.tile_pool(name="opair", bufs=1))
    rc_pool = ctx.enter_context(tc.tile_pool(name="rc", bufs=8))
    # compact triangular E storage: chunk kc holds columns [kc*128, S),
    # resident for the whole head (PV is q-tile-outer).  Early chunks are
    # double-buffered so the next head's scores can run several chunks
    # ahead of this head's last PV q-tiles (pipeline skew).
    e_pool = ctx.enter_context(tc.tile_pool(name="epool", bufs=1))
    e_pool2 = ctx.enter_context(tc.tile_pool(name="epool2", bufs=2))
    em_pool = ctx.enter_context(tc.tile_pool(name="em", bufs=9))
    ob_pool = ctx.enter_context(tc.tile_pool(name="ob", bufs=3))

    qt = [qkt_pool.tile([128, S], BF16, tag=f"qt{j}", name=f"qt{j}") for j in range(NP)]
    kt = [qkt_pool.tile([128, S], BF16, tag=f"kt{j}", name=f"kt{j}") for j in range(NP)]
    v3 = [v3_pool.tile([128, HPC * 65], BF16, tag=f"v3_{st}", name=f"v3_{st}")
          for st in range(NS)]
    oo = [oo_pool.tile([128, S], BF16, tag=f"oo{j}", name=f"oo{j}") for j in range(NP)]
    opair = [op_pool.tile([128, 128], BF16, tag=f"opr{t}", name=f"opr{t}")
             for t in range(NS)]

    # single PSUM footprint for the whole kernel (8 banks):
    #   pst: 2x[128,1024] wide score slots (also recycled by the out-proj)
    #   psm: 2x one-bank slots - narrow late-kc scores, PE transposes,
    #        and drip-fed projection groups (deepens the exp pipeline)
    #   pop: 2 rotating PV accumulator banks (each accumulation group must
    #        own a full bank: start= clears the whole bank)
    pst = ctx.enter_context(tc.tile_pool(name="pst", bufs=2, space="PSUM"))
    psm = ctx.enter_context(tc.tile_pool(name="psm", bufs=2, space="PSUM"))
    pop = ctx.enter_context(tc.tile_pool(name="pop", bufs=2, space="PSUM"))

    # ---- projection group emitters ----
    def emit_v_group(st, pool=None, tag="st"):
        ps = (pool or pst).tile([128, HPC * DH], F32, tag=tag, name="psv")
        for dc in range(NDC):
            nc.tensor.matmul(
                ps[:], xt[dc][:, st * 128:(st + 1) * 128], wall[dc][:, 1024:1536],
                start=(dc == 0), stop=(dc == NDC - 1))
        v_view = v3[st][:].rearrange("p (h e) -> p h e", h=HPC)
        nc.vector.tensor_add(
            v_view[:, :, 0:DH],
            ps[:].rearrange("p (h e) -> p h e", h=HPC),
            bvb[:].rearrange("p (h e) -> p h e", h=HPC))
        nc.vector.memset(v_view[:, :, DH:65], 1.0)

    def emit_qk_group(mt, nb, pool=None, tag="st"):
        t, j = mt // NP, mt % NP
        dst = qt[j] if t == 0 else kt[j]
        ps = (pool or pst).tile([128, 512], F32, tag=tag, name="psqk")
        for dc in range(NDC):
            nc.tensor.matmul(
                ps[:], wall[dc][:, mt * 128:(mt + 1) * 128],
                xt[dc][:, nb * 512:(nb + 1) * 512],
                start=(dc == 0), stop=(dc == NDC - 1))
        nc.vector.tensor_scalar_add(
            dst[:, nb * 512:(nb + 1) * 512], ps[:], bqk[:, mt:mt + 1])

    # upfront: just enough for head 0 to start, spread across all idle
    # psum bank tags (attention hasn't claimed them yet)
    emit_qk_group(4, 0, tag="st")
    emit_qk_group(0, 0, tag="st")
    emit_qk_group(0, 1, pool=psm, tag="sm")
    emit_qk_group(0, 2, pool=psm, tag="sm")
    emit_qk_group(0, 3, pool=pop, tag="pv")
    emit_v_group(0, pool=pop, tag="pv")
    emit_v_group(1, tag="st")

    # everything else drip-feeds into the attention instruction stream,
    # paced so each projection lands just before its deadline (kt/qt pair p
    # before head 2p; v3[st] early in head 0) and fills PE idle in the
    # otherwise exp-paced middle heads.  Injected groups use the "tp" bank
    # so both score slots stay free to run the exp pipeline ahead.
    deferred = [("qk", 4, 1), ("v", 2, 0), ("qk", 4, 2), ("v", 3, 0),
                ("qk", 4, 3), ("v", 4, 0)]
    deferred += [("v", st, 0) for st in range(5, NS)]
    # per pair jj (heads 2jj/2jj+1 start at iter 32*jj): qt groups must all
    # land just before the pair's first head; kt columns stream in with kc.
    for jj in (1, 2, 3):
        deferred += [("qk", jj, nb) for nb in range(NQ)]
        deferred += [("qk", NP + jj, nb) for nb in range(NQ)]
    deferred.reverse()  # pop() from the end
    # injection slots: iter -> count (deadline-aware, as late as legal so PE
    # filler lands inside the exp-paced attention middle)
    _sched = {i: 2 for i in range(1, 9)}
    _sched[9] = 1
    for base in (32, 64, 96):              # pair jj = base//32 starts at base
        for i in (-14, -11, -8, -5):       # qt groups: all before the pair
            _sched[base + i] = 1
        for i in (-4, 0, 4, 8):            # kt group nb=k needed by base+4k
            _sched[base + i] = 1

    def inject(n):
        for _ in range(n):
            if not deferred:
                return
            kind, a, b = deferred.pop()
            if kind == "qk":
                emit_qk_group(a, b, pool=psm, tag="sm")
            else:
                emit_v_group(a, pool=psm, tag="sm")

    def emit_scores(h, kc, e_of, em_of):
        """S^T chunk -> exp -> compact e (cols kc*128..S) + masked diag em."""
        j, po = h // 2, (h % 2) * 64
        c0 = kc * 128
        pool = e_pool2 if kc < 6 else e_pool
        e_t = pool.tile([128, S - c0], BF16, tag=f"e{kc}", name=f"e{kc}")
        seg0 = c0
        while seg0 < S:
            segw = min(1024 - seg0 % 1024, S - seg0)
            if kc >= 12:
                st_ps = psm.tile([128, 512], F32, tag="sm", name="sm")
            else:
                st_ps = pst.tile([128, 1024], F32, tag="st", name="st")
            sb = seg0 % 1024 if kc < 12 else seg0 % 512
            p0 = seg0
            while p0 < seg0 + segw:
                pw = min(512 - p0 % 512, seg0 + segw - p0)
                nc.tensor.matmul(
                    st_ps[:, p0 - seg0 + sb:p0 - seg0 + sb + pw],
                    kt[j][po:po + 64, kc * 128:(kc + 1) * 128],
                    qt[j][po:po + 64, p0:p0 + pw],
                    start=True, stop=True)
                p0 += pw
            nc.scalar.activation(
                e_t[:, seg0 - c0:seg0 - c0 + segw], st_ps[:, sb:sb + segw],
                AF.Exp, scale=0.125)
            seg0 += segw
        # causal mask of the diagonal 128x128 block into a separate tile
        # (gpsimd: idle engine) so off-diagonal PV matmuls don't wait on it
        em = em_pool.tile([128, 128], BF16, tag="em", name="em")
        nc.vector.tensor_mul(em[:], e_t[:, 0:128], tri[:])
        if DEBUG and h == 0 and kc == 0:
            nc.sync.dma_start(io[-1][4][0:128, :], e_t[:])
            nc.sync.dma_start(io[-1][4][128:256, 0:128], em[:])
        e_of[kc] = e_t
        em_of[kc] = em

    def emit_outproj(mts):
        for mt in mts:
            ps = pst.tile([128, D], F32, tag="st", name="psz")
            for nb in range(2):
                for jj in range(NP):
                    nc.tensor.matmul(
                        ps[:, nb * 512:(nb + 1) * 512],
                        oo[jj][:, mt * 128:(mt + 1) * 128],
                        wo[jj][:, nb * 512:(nb + 1) * 512],
                        start=(jj == 0), stop=(jj == NP - 1))
            ob = ob_pool.tile([128, D], BF16, tag="ob", name="ob")
            nc.scalar.copy(ob[:], ps[:])
            nc.sync.dma_start(out_d[mt * 128:(mt + 1) * 128, :], ob[:])

    def emit_pv(h, t, e_of, em_of):
        """PV for q-tile t: one short-lived accumulation group per bank,
        then normalize (and transpose once the head pair is complete)."""
        j, po = h // 2, (h % 2) * 64
        ps = pop.tile([128, 65], F32, tag="pv", name="pv")
        for kc in range(t + 1):
            lhsT = (em_of[kc][:] if kc == t
                    else e_of[kc][:, (t - kc) * 128:(t - kc + 1) * 128])
            nc.tensor.matmul(
                ps[:], lhsT, v3[kc][:, h * 65:(h + 1) * 65],
                start=(kc == 0), stop=(kc == t))
        r = rc_pool.tile([128, 1], F32, tag="rc", name="rc")
        nc.vector.reciprocal(r[:], ps[:, 64:65])
        nc.vector.tensor_scalar_mul(
            opair[t][:, po:po + 64], ps[:, 0:DH], r[:])
        if po:  # pair complete: transpose back to [he, q] for out-proj
            tps = psm.tile([128, 128], BF16, tag="sm", name="tps")
            nc.tensor.transpose(tps[:], opair[t][:], ident[:])
            if h == HPC - 1:  # ACT is idle in the tail; DVE is not
                nc.scalar.copy(oo[j][:, t * 128:(t + 1) * 128], tps[:])
                emit_outproj([t])  # all pairs done: out-proj for this q-tile
            else:
                nc.vector.tensor_copy(oo[j][:, t * 128:(t + 1) * 128], tps[:])

    # ---- attention, software-pipelined four k-chunks ahead ----
    from collections import deque
    pending = deque()
    kc_iter = 0
    e_of, em_of = [None] * NS, [None] * NS
    for h in range(HPC):
        for kc in range(NS):
            inject(_sched.get(kc_iter, 0))
            kc_iter += 1
            emit_scores(h, kc, e_of, em_of)
            pending.append((h, kc, list(e_of), list(em_of)))
            lim = 5 if h == HPC - 1 else 7
            while len(pending) > lim:
                emit_pv(*pending.popleft())
    while pending:
        emit_pv(*pending.popleft())

    if DEBUG:
        dbg_qt, dbg_kt, dbg_v3, dbg_oo, dbg_e, dbg_op = io[-1]
        for t in range(NS):
            nc.sync.dma_start(dbg_op[t * 128:(t + 1) * 128, :], opair[t][:])
        for jj in range(NP):
            nc.sync.dma_start(dbg_qt[jj * 128:(jj + 1) * 128, :], qt[jj][:])
            nc.sync.dma_start(dbg_kt[jj * 128:(jj + 1) * 128, :], kt[jj][:])
            nc.sync.dma_start(dbg_oo[jj * 128:(jj + 1) * 128, :], oo[jj][:])
        for st in range(NS):
            nc.sync.dma_start(dbg_v3[st * 128:(st + 1) * 128, :], v3[st][:])


def _build():
    nc = bacc.Bacc("TRN2", target_bir_lowering=False, debug=False)
    x_d = nc.dram_tensor("x_s", [D, S], BF16, kind="ExternalInput").ap()
    wall_d = nc.dram_tensor("wall", [D, 1536], BF16, kind="ExternalInput").ap()
    bqk_d = nc.dram_tensor("bqk", [128, 8], F32, kind="ExternalInput").ap()
    bvb_d = nc.dram_tensor("bvb", [128, HPC * DH], F32, kind="ExternalInput").ap()
    wo_d = nc.dram_tensor("wo", [HPC * DH, D], BF16, kind="ExternalInput").ap()
    tri_d = nc.dram_tensor("tri", [128, 128], BF16, kind="ExternalInput").ap()
    id_d = nc.dram_tensor("ident", [128, 128], BF16, kind="ExternalInput").ap()
    out_d = nc.dram_tensor("out_s", [S, D], BF16, kind="ExternalOutput").ap()
    io = [x_d, wall_d, bqk_d, bvb_d, wo_d, tri_d, id_d, out_d]
    if DEBUG:
        dbg = (nc.dram_tensor("dbg_qt", [512, S], BF16, kind="ExternalOutput").ap(),
               nc.dram_tensor("dbg_kt", [512, S], BF16, kind="ExternalOutput").ap(),
               nc.dram_tensor("dbg_v3", [NS * 128, HPC * 65], BF16, kind="ExternalOutput").ap(),
               nc.dram_tensor("dbg_oo", [512, S], BF16, kind="ExternalOutput").ap(),
               nc.dram_tensor("dbg_e", [256, S], BF16, kind="ExternalOutput").ap(),
               nc.dram_tensor("dbg_op", [NS * 128, 128], BF16, kind="ExternalOutput").ap())
        io.append(dbg)
    with tile.TileContext(nc) as tc:
        with ExitStack() as ctx:
            _emit(ctx, tc, io)
    nc.compile()
    return nc


_NC = None


def _get_nc():
    global _NC
    if _NC is None:
        _NC = _build()
    return _NC


def _host_inputs(x, w_qkv, b_qkv, w_out):
    """Per-head-group shared weight arrays + per-core x."""
    import ml_dtypes
    maps = []
    hg_arrs = []
    for hg in range(2):
        hs = slice(hg * HPC, (hg + 1) * HPC)
        wq = np.asarray(w_qkv[:, 0, hs, :]).reshape(D, HPC * DH)
        wk = np.asarray(w_qkv[:, 1, hs, :]).reshape(D, HPC * DH)
        wv = np.asarray(w_qkv[:, 2, hs, :]).reshape(D, HPC * DH)
        wall = np.concatenate([wq, wk, wv], axis=1).astype(ml_dtypes.bfloat16)
        bq = np.asarray(b_qkv[0, hs, :]).reshape(HPC * DH)
        bk = np.asarray(b_qkv[1, hs, :]).reshape(HPC * DH)
        bqk = np.zeros((128, 8), np.float32)
        for mt in range(8):
            t, j = mt // NP, mt % NP
            src = bq if t == 0 else bk
            bqk[:, mt] = src[j * 128:(j + 1) * 128]
        bvb = np.broadcast_to(
            np.asarray(b_qkv[2, hs, :]).reshape(1, HPC * DH), (128, HPC * DH)
        ).astype(np.float32)
        wo = np.asarray(w_out[hs]).reshape(HPC * DH, D).astype(ml_dtypes.bfloat16)
        trim = (np.arange(128)[None, :] >= np.arange(128)[:, None]).astype(
            ml_dtypes.bfloat16)
        idm = np.eye(128, dtype=ml_dtypes.bfloat16)
        hg_arrs.append(dict(wall=wall, bqk=bqk, bvb=bvb, wo=wo,
                            tri=trim, ident=idm))
    for c in range(8):
        b, hg = c % B, c // B
        m = dict(hg_arrs[hg])
        m["x_s"] = np.ascontiguousarray(
            np.asarray(x[b]).astype(ml_dtypes.bfloat16).T)
        maps.append(m)
    return maps


def _run(inputs, trace=False, tmpdir=None):
    nc = _get_nc()
    in_maps = _host_inputs(inputs["x"], inputs["w_qkv"], inputs["b_qkv"],
                           inputs["w_out"])
    res = bass_utils.run_bass_kernel_spmd(
        nc, in_maps, core_ids=list(range(8)), trace=trace, tmpdir=tmpdir)
    b_out = np.asarray(inputs["b_out"], dtype=np.float32)
    out = np.empty((B, S, D), np.float32)
    for b in range(B):
        out[b] = (res.results[b]["out_s"].astype(np.float32)
                  + res.results[b + B]["out_s"].astype(np.float32)
                  + b_out[None, :])
    return out, res


def kernel(**inputs) -> np.ndarray:
    out, _ = _run(inputs, trace=False)
    return out
